# revision 14
# baseline (speedup 1.0000x reference)
"""DualAttentionEncoderBlock Trainium2 Bass kernel.

Sharding: 8 cores = 4 batches x 2 token-halves + pair-wise collective.
Core (b, p) computes output tokens [512p, 512p+512) of batch b.
  - chan branch is HEAD-SPLIT across the pair: core p computes chan heads
    [8p, 8p+8) (projections, attention, softmax) and the out-projection
    partial over its 512 embed rows for ALL 1024 tokens; a pair-wise
    ReduceScatter(add) sums the partials and hands each core its own
    token half. The collective flies while the core runs the whole seq
    branch, so its latency is hidden.
  - seq branch: q-proj for its tokens, full K/V, rel-bias softmax
    attention, out-proj for its tokens (no collective).
  - FFN + final LN token-sliced.
Host assembles the 8 x (512, 768) outputs into (4, 1024, 768).

All matmul operands are bf16 (fp32 PSUM accumulation). Rel-pos bias is
applied as exp(s+b) = exp(s) * expb with host-precomputed exp(bias)
strips and a bf16 DVE multiply. Softmax sums come from a ones-column in
v_aug; per-head DVE reciprocal, gpsimd partition broadcast, DVE
multiply.
"""
import os

os.environ.setdefault("JAX_COMPILATION_CACHE_DIR", "/tmp/jax_bass_cache")

import numpy as np
import ml_dtypes

import concourse.bass as bass
import concourse.bacc as bacc
import concourse.tile as tile
import concourse.mybir as mybir
from concourse.bass_utils import run_bass_kernel_spmd

F32 = mybir.dt.float32
BF16 = mybir.dt.bfloat16
AF = mybir.ActivationFunctionType
SUB = mybir.AluOpType.subtract
MULT = mybir.AluOpType.mult

L = 1024
D = 768
H = 16
HD = 48
HDP = 64
LH = 512
HC = 64
F = 1536
MAXD = 1024
LN_EPS = 1e-5
RELB_W = 1408
NKD = D // 128    # 6
NKL = L // 128    # 8
NLT = LH // 128   # 4
NFT = F // 128    # 12

_CACHE = {}


def _bcast_row(ap, width, parts=128):
    return bass.AP(tensor=ap.tensor, offset=ap.offset, ap=[[0, parts], [1, width]])


def build(skip_affine=False):
    nc = bacc.Bacc("TRN2", target_bir_lowering=False, debug=False, num_devices=8)

    def inp(name, shape, dtype=BF16):
        return nc.dram_tensor(name, shape, dtype, kind="ExternalInput").ap()

    x = inp("x", (L, D))
    xT = inp("xT", (D, L))
    xqT = inp("xqT", (D, LH))
    xq_seq = inp("xq_seq", (LH, D), F32)
    xq_chan = inp("xq_chan", (LH, D), F32)
    wqkT = inp("wqkT", (D, 2 * MAXD))
    bqkp = inp("bqkp", (2 * MAXD, 1), F32)
    wvT = inp("wvT", (D, 80 * H))
    bv_row = inp("bv_row", (1, 80 * H), F32)
    woT = inp("woT", (MAXD, D))
    relb = inp("relb", (H, 128, RELB_W), BF16)
    wiT = inp("wiT", (L, MAXD))            # per-core head-group q|k in-proj
    cbqk = inp("cbqk", (MAXD, 1), F32)     # per-core head-group q|k bias
    wvcT = inp("wvcT", (L, 520))           # per-core head-group v (augmented)
    cvb_row = inp("cvb_row", (1, 520), F32)
    woutT = inp("woutT", (LH, L))          # per-core: its 512 embed rows, all l
    w1T = inp("w1T", (F, F))
    b1col = inp("b1col", (F, 1), F32)
    w2T = inp("w2T", (F, D))
    b2_row = inp("b2_row", (1, D), F32)
    g_seq_row = inp("g_seq_row", (1, D), F32)
    b_seq_row = inp("b_seq_row", (1, D), F32)
    g_chan_row = inp("g_chan_row", (1, D), F32)
    b_chan_row = inp("b_chan_row", (1, D), F32)
    g_ffn_row = inp("g_ffn_row", (1, D), F32)
    b_ffn_row = inp("b_ffn_row", (1, D), F32)
    ident16_in = inp("ident16_in", (128, 128), BF16)

    copart = nc.dram_tensor("copart", (L, D), F32)      # chan out-proj partial
    copartRS = nc.dram_tensor("copartRS", (LH, D), F32)  # pair-reduced own half
    out_d = nc.dram_tensor("out", (LH, D), F32, kind="ExternalOutput").ap()

    with tile.TileContext(nc) as tc:
        with (
            nc.allow_low_precision(reason="bf16 matmul operands, fp32 accum"),
            tc.tile_pool(name="smalls", bufs=1) as smalls,
            tc.tile_pool(name="resid", bufs=1) as resid,
            tc.tile_pool(name="lnrow", bufs=1) as lnrow,
            tc.tile_pool(name="lntmp", bufs=3 if skip_affine else 2) as lntmp,
            tc.tile_pool(name="xnat", bufs=1) as xnat,
        ):
            ident16 = smalls.tile([128, 128], BF16)
            nc.sync.dma_start(out=ident16, in_=ident16_in)
            eps_t = smalls.tile([128, 1], F32)
            nc.vector.memset(eps_t, LN_EPS)

            xseq_sb = resid.tile([128, NLT, D], BF16)
            xchan_sb = resid.tile([128, NLT, D], BF16)

            x_sb = xnat.tile([128, NKL, D], BF16)
            for kt in range(NKL):
                nc.sync.dma_start(
                    out=x_sb[:, kt, :], in_=x[kt * 128:(kt + 1) * 128, :]
                )

            def layernorm(t_sb, g_row, b_row, dst_ap):
                stats = lntmp.tile([128, 3, 6], F32, tag="ln_stats")
                for sg in range(3):
                    nc.vector.bn_stats(
                        out=stats[:, sg, :], in_=t_sb[:, sg * 256:(sg + 1) * 256]
                    )
                mv = lntmp.tile([128, 2], F32, tag="ln_mv")
                nc.vector.bn_aggr(out=mv, in_=stats)
                nc.scalar.activation(
                    out=mv[:, 1:2], in_=mv[:, 1:2], func=AF.Sqrt, bias=eps_t
                )
                nc.vector.reciprocal(out=mv[:, 1:2], in_=mv[:, 1:2])
                if skip_affine:
                    nc.vector.tensor_scalar(
                        out=dst_ap, in0=t_sb, scalar1=mv[:, 0:1],
                        scalar2=mv[:, 1:2], op0=SUB, op1=MULT,
                    )
                    return
                z = lntmp.tile([128, D], F32, tag="ln_z")
                nc.vector.tensor_scalar(
                    out=z, in0=t_sb, scalar1=mv[:, 0:1], scalar2=mv[:, 1:2],
                    op0=SUB, op1=MULT,
                )
                gb = lnrow.tile([128, D], F32, tag="ln_g")
                nc.sync.dma_start(out=gb, in_=_bcast_row(g_row, D))
                bb = lnrow.tile([128, D], F32, tag="ln_b")
                nc.sync.dma_start(out=bb, in_=_bcast_row(b_row, D))
                nc.vector.tensor_mul(out=z, in0=z, in1=gb)
                nc.vector.tensor_add(out=dst_ap, in0=z, in1=bb)

            # ======== CHAN branch: this core's 8 heads only ========
            with tc.tile_pool(name="chanot", bufs=1) as chanot:
                cbqk_sb = smalls.tile([128, 8, 1], F32)
                nc.sync.dma_start(
                    out=cbqk_sb, in_=cbqk.rearrange("(t p) o -> p t o", p=128)
                )
                OcT_sb = chanot.tile([128, 4, D], BF16)

                with (
                    tc.tile_pool(name="chacts", bufs=1) as chacts,
                    tc.tile_pool(name="wD", bufs=6) as wD,
                ):
                    qcT_sb = chacts.tile([128, 4, D], BF16)
                    kcT_sb = chacts.tile([128, 4, D], BF16)
                    vaugc_sb = chacts.tile([128, NKD, 520], BF16)
                    cvb8 = smalls.tile([128, 520], F32)

                    # q_c^T / k_c^T for this core's head group
                    with tc.tile_pool(name="psDqk", bufs=4, space="PSUM") as psDqk:
                        for qk in range(2):
                            pss = [
                                psDqk.tile([128, D], F32, tag="qkc", name=f"psqkc{i}")
                                for i in range(4)
                            ]
                            base = qk * 512
                            for kt in range(NKL):
                                w_t = wD.tile([128, 512], BF16, tag="wi")
                                nc.sync.dma_start(
                                    out=w_t,
                                    in_=wiT[
                                        kt * 128:(kt + 1) * 128, base:base + 512
                                    ],
                                )
                                for mi in range(4):
                                    for n0, n1 in ((0, 512), (512, D)):
                                        nc.tensor.matmul(
                                            pss[mi][:, n0:n1],
                                            w_t[:, mi * 128:(mi + 1) * 128],
                                            x_sb[:, kt, n0:n1],
                                            start=(kt == 0),
                                            stop=(kt == NKL - 1),
                                        )
                            dsts = qcT_sb if qk == 0 else kcT_sb
                            for mi in range(4):
                                nc.vector.tensor_scalar_add(
                                    out=dsts[:, mi, :], in0=pss[mi],
                                    scalar1=cbqk_sb[:, qk * 4 + mi, :],
                                )

                    # v_aug_c (520 = 8*65 cols incl ones)
                    nc.sync.dma_start(out=cvb8, in_=_bcast_row(cvb_row, 520))
                    with tc.tile_pool(name="psDv", bufs=2, space="PSUM") as psDv:
                        for g in range(3):
                            pss = [
                                psDv.tile([128, 520], F32, tag="vc", name=f"psvc{i}")
                                for i in range(2)
                            ]
                            for kt in range(NKL):
                                w_t = wD.tile([128, 520], BF16, tag="wvc")
                                nc.sync.dma_start(
                                    out=w_t,
                                    in_=wvcT[kt * 128:(kt + 1) * 128, :],
                                )
                                for mi in range(2):
                                    mtd = g * 2 + mi
                                    for n0, n1 in ((0, 512), (512, 520)):
                                        nc.tensor.matmul(
                                            pss[mi][:, n0:n1],
                                            x_sb[:, kt, mtd * 128:(mtd + 1) * 128],
                                            w_t[:, n0:n1],
                                            start=(kt == 0),
                                            stop=(kt == NKL - 1),
                                        )
                            for mi in range(2):
                                nc.vector.tensor_add(
                                    out=vaugc_sb[:, g * 2 + mi, :],
                                    in0=pss[mi], in1=cvb8,
                                )

                    # chan attention for the 8 local heads
                    with (
                        tc.tile_pool(name="scexp", bufs=6) as scexp,
                        tc.tile_pool(name="psSC", bufs=2, space="PSUM") as psSC,
                        tc.tile_pool(name="psOC", bufs=2, space="PSUM") as psOC,
                    ):
                        for hh in range(8):
                            hp = 64 * (hh % 2)
                            ht = hh // 2
                            oc_ps = psOC.tile([65, D], F32, tag="oc")
                            pendc = None
                            for d0 in range(NKD):
                                sc_ps = psSC.tile([128, D], F32, tag="sc")
                                for n0, n1 in ((0, 512), (512, D)):
                                    nc.tensor.matmul(
                                        sc_ps[:, n0:n1],
                                        kcT_sb[
                                            hp:hp + HC, ht,
                                            d0 * 128:(d0 + 1) * 128,
                                        ],
                                        qcT_sb[hp:hp + HC, ht, n0:n1],
                                        start=True,
                                        stop=True,
                                    )
                                scatt = scexp.tile([128, D], BF16, tag="scatt")
                                nc.scalar.activation(
                                    out=scatt, in_=sc_ps, func=AF.Exp
                                )
                                if pendc is not None:
                                    pd0, pscatt = pendc
                                    for n0, n1 in ((0, 512), (512, D)):
                                        nc.tensor.matmul(
                                            oc_ps[:, n0:n1],
                                            vaugc_sb[:, pd0, 65 * hh:65 * hh + 65],
                                            pscatt[:, n0:n1],
                                            start=(pd0 == 0),
                                            stop=False,
                                        )
                                pendc = (d0, scatt)
                            pd0, pscatt = pendc
                            for n0, n1 in ((0, 512), (512, D)):
                                nc.tensor.matmul(
                                    oc_ps[:, n0:n1],
                                    vaugc_sb[:, pd0, 65 * hh:65 * hh + 65],
                                    pscatt[:, n0:n1],
                                    start=(pd0 == 0),
                                    stop=True,
                                )
                            rsc = scexp.tile([1, D], F32, tag="rsc")
                            # 1/x = exp(-ln x) on ACT: frees DVE, which
                            # paces this phase; sums are ~1e3 so Ln is safe
                            nc.scalar.activation(
                                out=rsc, in_=oc_ps[64:65, :], func=AF.Ln
                            )
                            nc.scalar.activation(
                                out=rsc, in_=rsc, func=AF.Exp, scale=-1.0
                            )
                            bcc_sb = scexp.tile([HC, D], F32, tag="bcc_sb")
                            nc.gpsimd.partition_broadcast(bcc_sb, rsc)
                            nc.vector.tensor_mul(
                                out=OcT_sb[hp:hp + HC, ht, :],
                                in0=oc_ps[0:HC, :], in1=bcc_sb,
                            )

                # chan out-proj PARTIAL over this core's 512 embed rows,
                # for ALL 1024 tokens; pair ReduceScatter sums and scatters
                with (
                    tc.tile_pool(name="costg", bufs=1) as costg,
                    tc.tile_pool(name="wDo", bufs=4) as wDo,
                    tc.tile_pool(name="psDo", bufs=4, space="PSUM") as psDo,
                ):
                    stage = costg.tile([128, NKL, D], F32)
                    for P in range(2):
                        pss = [
                            psDo.tile([128, D], F32, tag="opc", name=f"psopc{i}")
                            for i in range(4)
                        ]
                        for kt in range(4):
                            w_t = wDo.tile([128, MAXD], BF16, tag="wout")
                            nc.sync.dma_start(
                                out=w_t, in_=woutT[kt * 128:(kt + 1) * 128, :]
                            )
                            for li in range(4):
                                lt = P * 4 + li
                                for n0, n1 in ((0, 512), (512, D)):
                                    nc.tensor.matmul(
                                        pss[li][:, n0:n1],
                                        w_t[:, lt * 128:(lt + 1) * 128],
                                        OcT_sb[:, kt, n0:n1],
                                        start=(kt == 0),
                                        stop=(kt == 3),
                                    )
                        for li in range(4):
                            nc.vector.tensor_copy(
                                out=stage[:, P * 4 + li, :], in_=pss[li]
                            )
                    nc.sync.dma_start(
                        out=copart.ap().rearrange("(t p) n -> p t n", p=128),
                        in_=stage,
                    )
                    nc.gpsimd.collective_compute(
                        "ReduceScatter",
                        mybir.AluOpType.add,
                        replica_groups=[[0, 1], [2, 3], [4, 5], [6, 7]],
                        ins=[copart.ap()],
                        outs=[copartRS.ap()],
                    )

            # ======== SEQ A: q^T, k^T, v_aug ========
            with tc.tile_pool(name="seqqkv", bufs=1) as seqqkv:
                qT_sb = seqqkv.tile([128, 8, LH], BF16)
                kT_sb = seqqkv.tile([128, 8, L], BF16)
                vaug_sb = seqqkv.tile([128, NKL, 80 * H], BF16)

                with (
                    tc.tile_pool(name="xloadA", bufs=1) as xloadA,
                    tc.tile_pool(name="wA", bufs=5) as wA,
                ):
                    xT_sb = xloadA.tile([128, NKD, L], BF16)
                    xqT_sb = xloadA.tile([128, NKD, LH], BF16)
                    bqkp_sb = smalls.tile([128, 16, 1], F32)
                    bvb = smalls.tile([128, 80 * H], F32)

                    with tc.tile_pool(name="psA", bufs=8, space="PSUM") as psA:
                        pss = [
                            psA.tile([128, LH], F32, tag="ps", name=f"psq{i}")
                            for i in range(8)
                        ]
                        for kt in range(NKD):
                            nc.sync.dma_start(
                                out=xqT_sb[:, kt, :],
                                in_=xqT[kt * 128:(kt + 1) * 128, :],
                            )
                            w_t = wA.tile([128, MAXD], BF16, tag="wq")
                            nc.sync.dma_start(
                                out=w_t, in_=wqkT[kt * 128:(kt + 1) * 128, 0:MAXD]
                            )
                            nc.sync.dma_start(
                                out=xT_sb[:, kt, :],
                                in_=xT[kt * 128:(kt + 1) * 128, :],
                            )
                            for mt in range(8):
                                nc.tensor.matmul(
                                    pss[mt],
                                    w_t[:, mt * 128:(mt + 1) * 128],
                                    xqT_sb[:, kt, :],
                                    start=(kt == 0),
                                    stop=(kt == NKD - 1),
                                )
                        nc.sync.dma_start(
                            out=bqkp_sb,
                            in_=bqkp.rearrange("(t p) o -> p t o", p=128),
                        )
                        nc.sync.dma_start(
                            out=bvb, in_=_bcast_row(bv_row, 80 * H)
                        )
                        for mt in range(8):
                            nc.vector.tensor_scalar_add(
                                out=qT_sb[:, mt, :], in0=pss[mt],
                                scalar1=bqkp_sb[:, mt, :],
                            )

                        for g in range(2):
                            pss = [
                                [
                                    psA.tile(
                                        [128, LH], F32, tag="ps",
                                        name=f"psk{i}_{nq}",
                                    )
                                    for nq in range(2)
                                ]
                                for i in range(4)
                            ]
                            for kt in range(NKD):
                                w_t = wA.tile([128, LH], BF16, tag="wk")
                                nc.sync.dma_start(
                                    out=w_t,
                                    in_=wqkT[
                                        kt * 128:(kt + 1) * 128,
                                        MAXD + g * 512:MAXD + (g + 1) * 512,
                                    ],
                                )
                                for mi in range(4):
                                    for nq in range(2):
                                        nc.tensor.matmul(
                                            pss[mi][nq],
                                            w_t[:, mi * 128:(mi + 1) * 128],
                                            xT_sb[:, kt, nq * 512:(nq + 1) * 512],
                                            start=(kt == 0),
                                            stop=(kt == NKD - 1),
                                        )
                            for mi in range(4):
                                mt = g * 4 + mi
                                for nq in range(2):
                                    nc.vector.tensor_scalar_add(
                                        out=kT_sb[:, mt, nq * 512:(nq + 1) * 512],
                                        in0=pss[mi][nq],
                                        scalar1=bqkp_sb[:, 8 + mt, :],
                                    )

                        for c in range(4):
                            pss = [
                                psA.tile([128, 320], F32, tag="ps", name=f"psv{i}")
                                for i in range(8)
                            ]
                            for kt in range(NKD):
                                w_t = wA.tile([128, 320], BF16, tag="wv")
                                nc.sync.dma_start(
                                    out=w_t,
                                    in_=wvT[
                                        kt * 128:(kt + 1) * 128,
                                        c * 320:(c + 1) * 320,
                                    ],
                                )
                                for mtk in range(8):
                                    nc.tensor.matmul(
                                        pss[mtk],
                                        xT_sb[:, kt, mtk * 128:(mtk + 1) * 128],
                                        w_t,
                                        start=(kt == 0),
                                        stop=(kt == NKD - 1),
                                    )
                            for mtk in range(8):
                                nc.vector.tensor_add(
                                    out=vaug_sb[:, mtk, c * 320:(c + 1) * 320],
                                    in0=pss[mtk],
                                    in1=bvb[:, c * 320:(c + 1) * 320],
                                )

                # ======== CHAN consume: reduce-scattered half + LN ========
                with tc.tile_pool(name="cocons", bufs=1) as cocons:
                    stage2 = cocons.tile([128, NLT, D], F32)
                    nc.sync.dma_start(
                        out=stage2,
                        in_=copartRS.ap().rearrange("(t p) n -> p t n", p=128),
                    )
                    xqchan_sb = cocons.tile([128, NLT, D], F32)
                    nc.sync.dma_start(
                        out=xqchan_sb,
                        in_=xq_chan.rearrange("(t p) n -> p t n", p=128),
                    )
                    for lt in range(NLT):
                        t_sb = lntmp.tile([128, D], F32, tag="ln_t")
                        nc.vector.tensor_add(
                            out=t_sb, in0=stage2[:, lt, :],
                            in1=xqchan_sb[:, lt, :],
                        )
                        layernorm(
                            t_sb, g_chan_row, b_chan_row, xchan_sb[:, lt, :]
                        )

                # ======== SEQ B: attention ========
                with tc.tile_pool(name="seqot", bufs=1) as seqot:
                    OT_sb = seqot.tile([128, NKL, LH], BF16)
                    with (
                        tc.tile_pool(name="relbp", bufs=4) as relbp,
                        tc.tile_pool(name="sexp", bufs=8) as sexp,
                        tc.tile_pool(name="otn", bufs=4) as otn,
                        tc.tile_pool(name="psS", bufs=3, space="PSUM") as psS,
                        tc.tile_pool(name="psO", bufs=2, space="PSUM") as psO,
                    ):
                        for hpair in range(H // 2):
                            h0, h1 = 2 * hpair, 2 * hpair + 1
                            ht = hpair
                            strips = []
                            for h in (h0, h1):
                                strip = relbp.tile(
                                    [128, RELB_W], BF16, tag="strip",
                                    name=f"strip{h % 2}",
                                )
                                nc.sync.dma_start(out=strip, in_=relb[h])
                                strips.append(strip)
                            o_pss = [
                                psO.tile([65, LH], F32, tag="o", name=f"ops{i}")
                                for i in range(2)
                            ]
                            pend = None
                            for kd in range(4):
                                s_pss = [
                                    psS.tile(
                                        [128, 2 * LH], F32, tag="s",
                                        name=f"sps{i}",
                                    )
                                    for i in range(2)
                                ]
                                for i, hp in ((0, 0), (1, 64)):
                                    for j in range(2):
                                        k0 = 2 * kd + j
                                        nc.tensor.matmul(
                                            s_pss[i][:, j * LH:(j + 1) * LH],
                                            kT_sb[
                                                hp:hp + HD, ht,
                                                k0 * 128:(k0 + 1) * 128,
                                            ],
                                            qT_sb[hp:hp + HD, ht, :],
                                            start=True,
                                            stop=True,
                                        )
                                attns = []
                                for i in range(2):
                                    attn = sexp.tile(
                                        [128, 2 * LH], BF16, tag="attn",
                                        name=f"attn{i}",
                                    )
                                    nc.scalar.activation(
                                        out=attn, in_=s_pss[i], func=AF.Exp
                                    )
                                    for j in range(2):
                                        k0 = 2 * kd + j
                                        c0 = 896 - k0 * 128
                                        nc.vector.tensor_mul(
                                            out=attn[:, j * LH:(j + 1) * LH],
                                            in0=attn[:, j * LH:(j + 1) * LH],
                                            in1=strips[i][:, c0:c0 + LH],
                                        )
                                    attns.append(attn)
                                if pend is not None:
                                    pkd, pattns = pend
                                    for i, h in ((0, h0), (1, h1)):
                                        for j in range(2):
                                            pk0 = 2 * pkd + j
                                            nc.tensor.matmul(
                                                o_pss[i],
                                                vaug_sb[:, pk0, 80 * h:80 * h + 65],
                                                pattns[i][:, j * LH:(j + 1) * LH],
                                                start=(pk0 == 0),
                                                stop=False,
                                            )
                                pend = (kd, attns)
                            pkd, pattns = pend
                            for i, h in ((0, h0), (1, h1)):
                                for j in range(2):
                                    pk0 = 2 * pkd + j
                                    nc.tensor.matmul(
                                        o_pss[i],
                                        vaug_sb[:, pk0, 80 * h:80 * h + 65],
                                        pattns[i][:, j * LH:(j + 1) * LH],
                                        start=(pk0 == 0),
                                        stop=(pk0 == NKL - 1),
                                    )
                            for i, h in ((0, h0), (1, h1)):
                                hp = 64 * i
                                rs = otn.tile([1, LH], F32, tag="rs")
                                if i == 0:
                                    nc.scalar.activation(
                                        out=rs, in_=o_pss[i][64:65, :],
                                        func=AF.Ln,
                                    )
                                    nc.scalar.activation(
                                        out=rs, in_=rs, func=AF.Exp, scale=-1.0
                                    )
                                else:
                                    nc.vector.reciprocal(
                                        out=rs, in_=o_pss[i][64:65, :]
                                    )
                                bc_sb = otn.tile([HC, LH], F32, tag="bc_sb")
                                nc.gpsimd.partition_broadcast(bc_sb, rs)
                                nc.vector.tensor_mul(
                                    out=OT_sb[hp:hp + HDP, ht, :],
                                    in0=o_pss[i][0:HDP, :], in1=bc_sb,
                                )

                    # ======== SEQ C: out-proj + LN ========
                    with (
                        tc.tile_pool(name="xqs", bufs=1) as xqs,
                        tc.tile_pool(name="wC", bufs=6) as wC,
                        tc.tile_pool(name="psC", bufs=4, space="PSUM") as psC,
                    ):
                        xqseq_sb = xqs.tile([128, NLT, D], F32)
                        pss = [
                            psC.tile([128, D], F32, tag="op", name=f"psop{i}")
                            for i in range(NLT)
                        ]
                        for kt in range(NKL):
                            w_t = wC.tile([128, D], BF16, tag="wo")
                            nc.sync.dma_start(
                                out=w_t, in_=woT[kt * 128:(kt + 1) * 128, :]
                            )
                            for lt in range(NLT):
                                for n0, n1 in ((0, 512), (512, D)):
                                    nc.tensor.matmul(
                                        pss[lt][:, n0:n1],
                                        OT_sb[:, kt, lt * 128:(lt + 1) * 128],
                                        w_t[:, n0:n1],
                                        start=(kt == 0),
                                        stop=(kt == NKL - 1),
                                    )
                        nc.sync.dma_start(
                            out=xqseq_sb,
                            in_=xq_seq.rearrange("(t p) n -> p t n", p=128),
                        )
                        for lt in range(NLT):
                            t_sb = lntmp.tile([128, D], F32, tag="ln_t")
                            nc.vector.tensor_add(
                                out=t_sb, in0=pss[lt], in1=xqseq_sb[:, lt, :]
                            )
                            layernorm(
                                t_sb, g_seq_row, b_seq_row, xseq_sb[:, lt, :]
                            )

            # ======== FFN + final LN ========
            with (
                tc.tile_pool(name="ffn", bufs=1) as ffn,
                tc.tile_pool(name="wE", bufs=6) as wE,
            ):
                fT_sb = ffn.tile([128, NFT, LH], BF16)
                b1_sb = smalls.tile([128, NFT, 1], F32)
                nc.sync.dma_start(
                    out=b1_sb, in_=b1col.rearrange("(t p) o -> p t o", p=128)
                )
                hT_sb = ffn.tile([128, NFT, LH], BF16)
                pre_res = ffn.tile([128, NLT, D], F32)
                if not skip_affine:
                    b2b = smalls.tile([128, D], F32)
                    nc.sync.dma_start(out=b2b, in_=_bcast_row(b2_row, D))
                with tc.tile_pool(name="psE", bufs=8, space="PSUM") as psE:
                    for ct in range(NFT):
                        for lt in range(NLT):
                            src = (
                                xseq_sb[:, lt, ct * 128:(ct + 1) * 128]
                                if ct < 6
                                else xchan_sb[:, lt, (ct - 6) * 128:(ct - 5) * 128]
                            )
                            tp = psE.tile([128, LH], F32, tag="ps", name="tp")
                            nc.tensor.matmul(
                                tp[:, 0:128], src, ident16, start=True, stop=True
                            )
                            nc.vector.tensor_copy(
                                out=fT_sb[:, ct, lt * 128:(lt + 1) * 128],
                                in_=tp[:, 0:128],
                            )

                    for g in range(2):
                        pss = [
                            psE.tile([128, LH], F32, tag="ps", name=f"psh{i}")
                            for i in range(6)
                        ]
                        for kt in range(NFT):
                            w_t = wE.tile([128, D], BF16, tag="w1")
                            nc.sync.dma_start(
                                out=w_t,
                                in_=w1T[
                                    kt * 128:(kt + 1) * 128, g * D:(g + 1) * D
                                ],
                            )
                            for mi in range(6):
                                nc.tensor.matmul(
                                    pss[mi],
                                    w_t[:, mi * 128:(mi + 1) * 128],
                                    fT_sb[:, kt, :],
                                    start=(kt == 0),
                                    stop=(kt == NFT - 1),
                                )
                        for mi in range(6):
                            mt = g * 6 + mi
                            nc.scalar.activation(
                                out=hT_sb[:, mt, :], in_=pss[mi], func=AF.Relu,
                                bias=b1_sb[:, mt, :],
                            )

                    for lt in range(NLT):
                        nc.vector.tensor_add(
                            out=pre_res[:, lt, :],
                            in0=xseq_sb[:, lt, :],
                            in1=xchan_sb[:, lt, :],
                        )
                        if not skip_affine:
                            nc.vector.tensor_add(
                                out=pre_res[:, lt, :], in0=pre_res[:, lt, :],
                                in1=b2b,
                            )

                    pss = [
                        [
                            psE.tile(
                                [128, LH], F32, tag="ps", name=f"psfo{i}_{nq}"
                            )
                            for nq in range(2)
                        ]
                        for i in range(NLT)
                    ]
                    for kt in range(NFT):
                        w_t = wE.tile([128, D], BF16, tag="w2")
                        nc.sync.dma_start(
                            out=w_t, in_=w2T[kt * 128:(kt + 1) * 128, :]
                        )
                        for lt in range(NLT):
                            for nq, (n0, n1) in enumerate(((0, 512), (512, D))):
                                nc.tensor.matmul(
                                    pss[lt][nq][:, 0:n1 - n0],
                                    hT_sb[:, kt, lt * 128:(lt + 1) * 128],
                                    w_t[:, n0:n1],
                                    start=(kt == 0),
                                    stop=(kt == NFT - 1),
                                )
                    for lt in range(NLT):
                        t_sb = lntmp.tile([128, D], F32, tag="ln_t")
                        for nq, (n0, n1) in enumerate(((0, 512), (512, D))):
                            nc.vector.tensor_add(
                                out=t_sb[:, n0:n1],
                                in0=pss[lt][nq][:, 0:n1 - n0],
                                in1=pre_res[:, lt, n0:n1],
                            )
                        o_sb = lntmp.tile([128, D], F32, tag="ln_o")
                        layernorm(t_sb, g_ffn_row, b_ffn_row, o_sb)
                        nc.sync.dma_start(
                            out=out_d[lt * 128:(lt + 1) * 128, :], in_=o_sb
                        )

    nc.compile()
    return nc


def _prep_inputs(inputs):
    bf16 = ml_dtypes.bfloat16
    x = np.asarray(inputs["x"], dtype=np.float32)
    wq = np.asarray(inputs["wq"], dtype=np.float32)
    bq = np.asarray(inputs["bq"], dtype=np.float32)
    wk = np.asarray(inputs["wk"], dtype=np.float32)
    bk = np.asarray(inputs["bk"], dtype=np.float32)
    wv = np.asarray(inputs["wv"], dtype=np.float32)
    bv = np.asarray(inputs["bv"], dtype=np.float32)
    wo = np.asarray(inputs["wo"], dtype=np.float32)
    bo = np.asarray(inputs["bo"], dtype=np.float32)
    rel_bias = np.asarray(inputs["rel_bias"], dtype=np.float32)
    ciw = np.asarray(inputs["chan_in_w"], dtype=np.float32)
    cib = np.asarray(inputs["chan_in_b"], dtype=np.float32)
    cow = np.asarray(inputs["chan_out_w"], dtype=np.float32)
    cob = np.asarray(inputs["chan_out_b"], dtype=np.float32)
    w1 = np.asarray(inputs["ffn_w1"], dtype=np.float32)
    b1 = np.asarray(inputs["ffn_b1"], dtype=np.float32)
    w2 = np.asarray(inputs["ffn_w2"], dtype=np.float32)
    b2 = np.asarray(inputs["ffn_b2"], dtype=np.float32)

    sc_s = 1.0 / np.sqrt(np.float32(HD))
    sc_c = 1.0 / np.sqrt(np.float32(HC))

    wqT_pad = np.zeros((D, MAXD), np.float32)
    wkT_pad = np.zeros((D, MAXD), np.float32)
    bq_pad = np.zeros((MAXD,), np.float32)
    bk_pad = np.zeros((MAXD,), np.float32)
    for h in range(H):
        wqT_pad[:, HDP * h:HDP * h + HD] = (wq[HD * h:HD * h + HD, :] * sc_s).T
        wkT_pad[:, HDP * h:HDP * h + HD] = wk[HD * h:HD * h + HD, :].T
        bq_pad[HDP * h:HDP * h + HD] = bq[HD * h:HD * h + HD] * sc_s
        bk_pad[HDP * h:HDP * h + HD] = bk[HD * h:HD * h + HD]
    wqkT = np.ascontiguousarray(
        np.concatenate([wqT_pad, wkT_pad], axis=1).astype(bf16)
    )
    bqkp = np.ascontiguousarray(np.concatenate([bq_pad, bk_pad])[:, None])

    wvT_aug = np.zeros((D, 80 * H), np.float32)
    bv_row = np.zeros((1, 80 * H), np.float32)
    for h in range(H):
        wvT_aug[:, 80 * h:80 * h + HD] = wv[HD * h:HD * h + HD, :].T
        bv_row[0, 80 * h:80 * h + HD] = bv[HD * h:HD * h + HD]
        bv_row[0, 80 * h + HC] = 1.0
    wvT_aug = wvT_aug.astype(bf16)

    woT_pad = np.zeros((MAXD, D), np.float32)
    for h in range(H):
        woT_pad[HDP * h:HDP * h + HD, :] = wo[:, HD * h:HD * h + HD].T
    woT_pad = woT_pad.astype(bf16)

    q_w = ciw[0:L] * sc_c
    k_w = ciw[L:2 * L]
    v_w = ciw[2 * L:3 * L]
    cib_q = cib[0:L] * sc_c
    cib_k = cib[L:2 * L]

    w1T = np.ascontiguousarray(w1.T.astype(bf16))
    w2T = np.ascontiguousarray(w2.T.astype(bf16))
    owT = np.ascontiguousarray(cow.T)

    g1 = np.ascontiguousarray(np.asarray(inputs["g_seq"], np.float32)[None, :])
    b1r = np.ascontiguousarray(np.asarray(inputs["b_seq"], np.float32)[None, :])
    g2 = np.ascontiguousarray(np.asarray(inputs["g_chan"], np.float32)[None, :])
    b2r = np.ascontiguousarray(np.asarray(inputs["b_chan"], np.float32)[None, :])
    g3 = np.ascontiguousarray(np.asarray(inputs["g_ffn"], np.float32)[None, :])
    b3r = np.ascontiguousarray(np.asarray(inputs["b_ffn"], np.float32)[None, :])

    # exp(bias) strips: exp(s + b) = exp(s) * exp(b) applied on gpsimd
    exp_rel = np.exp(rel_bias)
    relb_p = []
    ii = np.arange(128)[:, None]
    ff = np.arange(RELB_W)[None, :]
    for p in range(2):
        idx = ii - ff + (1919 - 512 * p)
        np.clip(idx, 0, 2 * MAXD - 2, out=idx)
        relb_p.append(np.ascontiguousarray(
            exp_rel[idx, :].transpose(2, 0, 1).astype(bf16)
        ))

    # per-pair-half chan head-group tensors
    wiT_p, cbqk_p, wvcT_p, cvb_p, woutT_p = [], [], [], [], []
    for p in range(2):
        hsl = slice(512 * p, 512 * p + 512)
        wiT_p.append(np.ascontiguousarray(
            np.concatenate([q_w.T[:, hsl], k_w.T[:, hsl]], axis=1).astype(bf16)
        ))
        cbqk_p.append(np.ascontiguousarray(
            np.concatenate([cib_q[hsl], cib_k[hsl]])[:, None]
        ))
        wvc = np.zeros((L, 520), np.float32)
        cvb = np.zeros((1, 520), np.float32)
        for hh in range(8):
            h = 8 * p + hh
            wvc[:, 65 * hh:65 * hh + HC] = v_w[HC * h:HC * h + HC, :].T
            cvb[0, 65 * hh:65 * hh + HC] = cib[2 * L + HC * h:2 * L + HC * h + HC]
            cvb[0, 65 * hh + HC] = 1.0
        wvcT_p.append(wvc.astype(bf16))
        cvb_p.append(cvb)
        woutT_p.append(np.ascontiguousarray(owT[hsl, :].astype(bf16)))

    in_maps = []
    for core in range(8):
        b, p = core // 2, core % 2
        sl = slice(512 * p, 512 * p + 512)
        xb = x[b]
        m = {
            "x": np.ascontiguousarray(xb.astype(bf16)),
            "xT": np.ascontiguousarray(xb.T.astype(bf16)),
            "xqT": np.ascontiguousarray(xb[sl].T.astype(bf16)),
            "xq_seq": np.ascontiguousarray(xb[sl] + bo[None, :]),
            "xq_chan": np.ascontiguousarray(xb[sl] + cob[sl][:, None]),
            "wqkT": wqkT,
            "bqkp": bqkp,
            "wvT": wvT_aug,
            "bv_row": bv_row,
            "woT": woT_pad,
            "relb": relb_p[p],
            "wiT": wiT_p[p],
            "cbqk": cbqk_p[p],
            "wvcT": wvcT_p[p],
            "cvb_row": cvb_p[p],
            "woutT": woutT_p[p],
            "w1T": w1T,
            "b1col": np.ascontiguousarray(b1[:, None]),
            "w2T": w2T,
            "b2_row": np.ascontiguousarray(b2[None, :]),
            "g_seq_row": g1, "b_seq_row": b1r,
            "g_chan_row": g2, "b_chan_row": b2r,
            "g_ffn_row": g3, "b_ffn_row": b3r,
            "ident16_in": np.eye(128, dtype=bf16),
        }
        in_maps.append(m)
    return in_maps


def kernel(**inputs) -> np.ndarray:
    in_maps = _prep_inputs(inputs)
    skip = all(
        np.all(np.asarray(inputs[g]) == 1.0) for g in ("g_seq", "g_chan", "g_ffn")
    ) and all(
        np.all(np.asarray(inputs[b]) == 0.0)
        for b in ("b_seq", "b_chan", "b_ffn", "ffn_b2")
    )
    key = ("nc", skip)
    if key not in _CACHE:
        _CACHE[key] = build(skip_affine=skip)
    res = run_bass_kernel_spmd(_CACHE[key], in_maps, core_ids=list(range(8)))
    out = np.empty((4, L, D), np.float32)
    for core in range(8):
        b, p = core // 2, core % 2
        out[b, 512 * p:512 * p + 512, :] = res.results[core]["out"]
    return out


# revision 15
# speedup vs baseline: 1.0339x; 1.0339x over previous
"""DualAttentionEncoderBlock Trainium2 Bass kernel.

Sharding: 8 cores = 4 batches x 2 token-halves + pair-wise collective.
Core (b, p) computes output tokens [512p, 512p+512) of batch b.
  - chan branch is HEAD-SPLIT across the pair: core p computes chan heads
    [8p, 8p+8) (projections, attention, softmax) and the out-projection
    partial over its 512 embed rows for ALL 1024 tokens; a pair-wise
    ReduceScatter(add) sums the partials and hands each core its own
    token half. The collective flies while the core runs the whole seq
    branch, so its latency is hidden.
  - seq branch: q-proj for its tokens, full K/V, rel-bias softmax
    attention, out-proj for its tokens (no collective).
  - FFN + final LN token-sliced.
Host assembles the 8 x (512, 768) outputs into (4, 1024, 768).

All matmul operands are bf16 (fp32 PSUM accumulation). Rel-pos bias is
applied as exp(s+b) = exp(s) * expb with host-precomputed exp(bias)
strips and a bf16 DVE multiply. Softmax sums come from a ones-column in
v_aug; per-head DVE reciprocal, gpsimd partition broadcast, DVE
multiply.
"""
import os

os.environ.setdefault("JAX_COMPILATION_CACHE_DIR", "/tmp/jax_bass_cache")

import numpy as np
import ml_dtypes

import concourse.bass as bass
import concourse.bacc as bacc
import concourse.tile as tile
import concourse.mybir as mybir
from concourse.bass_utils import run_bass_kernel_spmd

F32 = mybir.dt.float32
BF16 = mybir.dt.bfloat16
AF = mybir.ActivationFunctionType
SUB = mybir.AluOpType.subtract
MULT = mybir.AluOpType.mult

L = 1024
D = 768
H = 16
HD = 48
HDP = 64
LH = 512
HC = 64
F = 1536
MAXD = 1024
LN_EPS = 1e-5
RELB_W = 1408
NKD = D // 128    # 6
NKL = L // 128    # 8
NLT = LH // 128   # 4
NFT = F // 128    # 12

_CACHE = {}


def _bcast_row(ap, width, parts=128):
    return bass.AP(tensor=ap.tensor, offset=ap.offset, ap=[[0, parts], [1, width]])


def build(skip_affine=False):
    nc = bacc.Bacc("TRN2", target_bir_lowering=False, debug=False, num_devices=8)

    def inp(name, shape, dtype=BF16):
        return nc.dram_tensor(name, shape, dtype, kind="ExternalInput").ap()

    x = inp("x", (L, D))
    xT = inp("xT", (D, L))
    xqT = inp("xqT", (D, LH))
    xq_seq = inp("xq_seq", (LH, D), F32)
    xq_chan = inp("xq_chan", (LH, D), F32)
    wqkT = inp("wqkT", (D, 2 * MAXD))
    bqkp = inp("bqkp", (2 * MAXD, 1), F32)
    wvT = inp("wvT", (D, 80 * H))
    bv_row = inp("bv_row", (1, 80 * H), F32)
    woT = inp("woT", (MAXD, D))
    relb = inp("relb", (H, 128, RELB_W), BF16)
    wiT = inp("wiT", (L, MAXD))            # per-core head-group q|k in-proj
    cbqk = inp("cbqk", (MAXD, 1), F32)     # per-core head-group q|k bias
    wvcT = inp("wvcT", (L, 520))           # per-core head-group v (augmented)
    cvb_row = inp("cvb_row", (1, 520), F32)
    woutT = inp("woutT", (LH, L))          # per-core: its 512 embed rows, all l
    w1T = inp("w1T", (F, F))
    b1col = inp("b1col", (F, 1), F32)
    w2T = inp("w2T", (F, D))
    b2_row = inp("b2_row", (1, D), F32)
    g_seq_row = inp("g_seq_row", (1, D), F32)
    b_seq_row = inp("b_seq_row", (1, D), F32)
    g_chan_row = inp("g_chan_row", (1, D), F32)
    b_chan_row = inp("b_chan_row", (1, D), F32)
    g_ffn_row = inp("g_ffn_row", (1, D), F32)
    b_ffn_row = inp("b_ffn_row", (1, D), F32)
    ident16_in = inp("ident16_in", (128, 128), BF16)

    copart = nc.dram_tensor("copart", (L, D), F32)      # chan out-proj partial
    copartRS = nc.dram_tensor("copartRS", (LH, D), F32)  # pair-reduced own half
    out_d = nc.dram_tensor("out", (LH, D), F32, kind="ExternalOutput").ap()

    with tile.TileContext(nc) as tc:
        with (
            nc.allow_low_precision(reason="bf16 matmul operands, fp32 accum"),
            tc.tile_pool(name="smalls", bufs=1) as smalls,
            tc.tile_pool(name="resid", bufs=1) as resid,
            tc.tile_pool(name="lnrow", bufs=1) as lnrow,
            tc.tile_pool(name="lntmp", bufs=3 if skip_affine else 2) as lntmp,
            tc.tile_pool(name="xnat", bufs=1) as xnat,
        ):
            ident16 = smalls.tile([128, 128], BF16)
            nc.sync.dma_start(out=ident16, in_=ident16_in)
            eps_t = smalls.tile([128, 1], F32)
            nc.vector.memset(eps_t, LN_EPS)

            xseq_sb = resid.tile([128, NLT, D], BF16)
            xchan_sb = resid.tile([128, NLT, D], BF16)

            x_sb = xnat.tile([128, NKL, D], BF16)
            for kt in range(NKL):
                nc.sync.dma_start(
                    out=x_sb[:, kt, :], in_=x[kt * 128:(kt + 1) * 128, :]
                )

            def layernorm(t_sb, g_row, b_row, dst_ap):
                stats = lntmp.tile([128, 3, 6], F32, tag="ln_stats")
                for sg in range(3):
                    nc.vector.bn_stats(
                        out=stats[:, sg, :], in_=t_sb[:, sg * 256:(sg + 1) * 256]
                    )
                mv = lntmp.tile([128, 2], F32, tag="ln_mv")
                nc.vector.bn_aggr(out=mv, in_=stats)
                nc.scalar.activation(
                    out=mv[:, 1:2], in_=mv[:, 1:2], func=AF.Sqrt, bias=eps_t
                )
                nc.vector.reciprocal(out=mv[:, 1:2], in_=mv[:, 1:2])
                if skip_affine:
                    nc.vector.tensor_scalar(
                        out=dst_ap, in0=t_sb, scalar1=mv[:, 0:1],
                        scalar2=mv[:, 1:2], op0=SUB, op1=MULT,
                    )
                    return
                z = lntmp.tile([128, D], F32, tag="ln_z")
                nc.vector.tensor_scalar(
                    out=z, in0=t_sb, scalar1=mv[:, 0:1], scalar2=mv[:, 1:2],
                    op0=SUB, op1=MULT,
                )
                gb = lnrow.tile([128, D], F32, tag="ln_g")
                nc.sync.dma_start(out=gb, in_=_bcast_row(g_row, D))
                bb = lnrow.tile([128, D], F32, tag="ln_b")
                nc.sync.dma_start(out=bb, in_=_bcast_row(b_row, D))
                nc.vector.tensor_mul(out=z, in0=z, in1=gb)
                nc.vector.tensor_add(out=dst_ap, in0=z, in1=bb)

            # ======== CHAN branch: this core's 8 heads only ========
            with tc.tile_pool(name="chanot", bufs=1) as chanot:
                cbqk_sb = smalls.tile([128, 8, 1], F32)
                nc.sync.dma_start(
                    out=cbqk_sb, in_=cbqk.rearrange("(t p) o -> p t o", p=128)
                )
                OcT_sb = chanot.tile([128, 4, D], BF16)

                with (
                    tc.tile_pool(name="chacts", bufs=1) as chacts,
                    tc.tile_pool(name="wD", bufs=6) as wD,
                ):
                    qcT_sb = chacts.tile([128, 4, D], BF16)
                    kcT_sb = chacts.tile([128, 4, D], BF16)
                    vaugc_sb = chacts.tile([128, NKD, 520], BF16)
                    cvb8 = smalls.tile([128, 520], F32)

                    # q_c^T / k_c^T for this core's head group
                    with tc.tile_pool(name="psDqk", bufs=4, space="PSUM") as psDqk:
                        for qk in range(2):
                            pss = [
                                psDqk.tile([128, D], F32, tag="qkc", name=f"psqkc{i}")
                                for i in range(4)
                            ]
                            base = qk * 512
                            for kt in range(NKL):
                                w_t = wD.tile([128, 512], BF16, tag="wi")
                                nc.sync.dma_start(
                                    out=w_t,
                                    in_=wiT[
                                        kt * 128:(kt + 1) * 128, base:base + 512
                                    ],
                                )
                                for mi in range(4):
                                    for n0, n1 in ((0, 512), (512, D)):
                                        nc.tensor.matmul(
                                            pss[mi][:, n0:n1],
                                            w_t[:, mi * 128:(mi + 1) * 128],
                                            x_sb[:, kt, n0:n1],
                                            start=(kt == 0),
                                            stop=(kt == NKL - 1),
                                        )
                            dsts = qcT_sb if qk == 0 else kcT_sb
                            for mi in range(4):
                                nc.vector.tensor_scalar_add(
                                    out=dsts[:, mi, :], in0=pss[mi],
                                    scalar1=cbqk_sb[:, qk * 4 + mi, :],
                                )

                    # v_aug_c (520 = 8*65 cols incl ones)
                    nc.sync.dma_start(out=cvb8, in_=_bcast_row(cvb_row, 520))
                    with tc.tile_pool(name="psDv", bufs=2, space="PSUM") as psDv:
                        for g in range(3):
                            pss = [
                                psDv.tile([128, 520], F32, tag="vc", name=f"psvc{i}")
                                for i in range(2)
                            ]
                            for kt in range(NKL):
                                w_t = wD.tile([128, 520], BF16, tag="wvc")
                                nc.sync.dma_start(
                                    out=w_t,
                                    in_=wvcT[kt * 128:(kt + 1) * 128, :],
                                )
                                for mi in range(2):
                                    mtd = g * 2 + mi
                                    for n0, n1 in ((0, 512), (512, 520)):
                                        nc.tensor.matmul(
                                            pss[mi][:, n0:n1],
                                            x_sb[:, kt, mtd * 128:(mtd + 1) * 128],
                                            w_t[:, n0:n1],
                                            start=(kt == 0),
                                            stop=(kt == NKL - 1),
                                        )
                            for mi in range(2):
                                nc.vector.tensor_add(
                                    out=vaugc_sb[:, g * 2 + mi, :],
                                    in0=pss[mi], in1=cvb8,
                                )

                    # chan attention for the 8 local heads
                    with (
                        tc.tile_pool(name="scexp", bufs=6) as scexp,
                        tc.tile_pool(name="psSC", bufs=2, space="PSUM") as psSC,
                        tc.tile_pool(name="psOC", bufs=2, space="PSUM") as psOC,
                    ):
                        for hh in range(8):
                            hp = 64 * (hh % 2)
                            ht = hh // 2
                            oc_ps = psOC.tile([65, D], F32, tag="oc")
                            pendc = None
                            for d0 in range(NKD):
                                sc_ps = psSC.tile([128, D], F32, tag="sc")
                                for n0, n1 in ((0, 512), (512, D)):
                                    nc.tensor.matmul(
                                        sc_ps[:, n0:n1],
                                        kcT_sb[
                                            hp:hp + HC, ht,
                                            d0 * 128:(d0 + 1) * 128,
                                        ],
                                        qcT_sb[hp:hp + HC, ht, n0:n1],
                                        start=True,
                                        stop=True,
                                    )
                                scatt = scexp.tile([128, D], BF16, tag="scatt")
                                nc.scalar.activation(
                                    out=scatt, in_=sc_ps, func=AF.Exp
                                )
                                if pendc is not None:
                                    pd0, pscatt = pendc
                                    for n0, n1 in ((0, 512), (512, D)):
                                        nc.tensor.matmul(
                                            oc_ps[:, n0:n1],
                                            vaugc_sb[:, pd0, 65 * hh:65 * hh + 65],
                                            pscatt[:, n0:n1],
                                            start=(pd0 == 0),
                                            stop=False,
                                        )
                                pendc = (d0, scatt)
                            pd0, pscatt = pendc
                            for n0, n1 in ((0, 512), (512, D)):
                                nc.tensor.matmul(
                                    oc_ps[:, n0:n1],
                                    vaugc_sb[:, pd0, 65 * hh:65 * hh + 65],
                                    pscatt[:, n0:n1],
                                    start=(pd0 == 0),
                                    stop=True,
                                )
                            rsc = scexp.tile([1, D], F32, tag="rsc")
                            # 1/x: alternate engines so neither ACT (exp
                            # FIFO) nor DVE paces the phase alone; Ln is
                            # safe, sums are ~1e3
                            if hh % 2 == 0:
                                nc.vector.reciprocal(
                                    out=rsc, in_=oc_ps[64:65, :]
                                )
                            else:
                                nc.scalar.activation(
                                    out=rsc, in_=oc_ps[64:65, :], func=AF.Ln
                                )
                                nc.scalar.activation(
                                    out=rsc, in_=rsc, func=AF.Exp, scale=-1.0
                                )
                            bcc_sb = scexp.tile([HC, D], F32, tag="bcc_sb")
                            nc.gpsimd.partition_broadcast(bcc_sb, rsc)
                            nc.vector.tensor_mul(
                                out=OcT_sb[hp:hp + HC, ht, :],
                                in0=oc_ps[0:HC, :], in1=bcc_sb,
                            )

                # chan out-proj PARTIAL over this core's 512 embed rows,
                # for ALL 1024 tokens; pair ReduceScatter sums and scatters
                with (
                    tc.tile_pool(name="costg", bufs=1) as costg,
                    tc.tile_pool(name="wDo", bufs=4) as wDo,
                    tc.tile_pool(name="psDo", bufs=4, space="PSUM") as psDo,
                ):
                    stage = costg.tile([128, NKL, D], F32)
                    for P in range(2):
                        pss = [
                            psDo.tile([128, D], F32, tag="opc", name=f"psopc{i}")
                            for i in range(4)
                        ]
                        for kt in range(4):
                            w_t = wDo.tile([128, MAXD], BF16, tag="wout")
                            nc.sync.dma_start(
                                out=w_t, in_=woutT[kt * 128:(kt + 1) * 128, :]
                            )
                            for li in range(4):
                                lt = P * 4 + li
                                for n0, n1 in ((0, 512), (512, D)):
                                    nc.tensor.matmul(
                                        pss[li][:, n0:n1],
                                        w_t[:, lt * 128:(lt + 1) * 128],
                                        OcT_sb[:, kt, n0:n1],
                                        start=(kt == 0),
                                        stop=(kt == 3),
                                    )
                        for li in range(4):
                            nc.vector.tensor_copy(
                                out=stage[:, P * 4 + li, :], in_=pss[li]
                            )
                    nc.sync.dma_start(
                        out=copart.ap().rearrange("(t p) n -> p t n", p=128),
                        in_=stage,
                    )
                    nc.gpsimd.collective_compute(
                        "ReduceScatter",
                        mybir.AluOpType.add,
                        replica_groups=[[0, 1], [2, 3], [4, 5], [6, 7]],
                        ins=[copart.ap()],
                        outs=[copartRS.ap()],
                    )

            # ======== SEQ A: q^T, k^T, v_aug ========
            with tc.tile_pool(name="seqqkv", bufs=1) as seqqkv:
                qT_sb = seqqkv.tile([128, 8, LH], BF16)
                kT_sb = seqqkv.tile([128, 8, L], BF16)
                vaug_sb = seqqkv.tile([128, NKL, 80 * H], BF16)

                with (
                    tc.tile_pool(name="xloadA", bufs=1) as xloadA,
                    tc.tile_pool(name="wA", bufs=5) as wA,
                ):
                    xT_sb = xloadA.tile([128, NKD, L], BF16)
                    xqT_sb = xloadA.tile([128, NKD, LH], BF16)
                    bqkp_sb = smalls.tile([128, 16, 1], F32)
                    bvb = smalls.tile([128, 80 * H], F32)

                    with tc.tile_pool(name="psA", bufs=8, space="PSUM") as psA:
                        pss = [
                            psA.tile([128, LH], F32, tag="ps", name=f"psq{i}")
                            for i in range(8)
                        ]
                        for kt in range(NKD):
                            nc.sync.dma_start(
                                out=xqT_sb[:, kt, :],
                                in_=xqT[kt * 128:(kt + 1) * 128, :],
                            )
                            w_t = wA.tile([128, MAXD], BF16, tag="wq")
                            nc.sync.dma_start(
                                out=w_t, in_=wqkT[kt * 128:(kt + 1) * 128, 0:MAXD]
                            )
                            nc.sync.dma_start(
                                out=xT_sb[:, kt, :],
                                in_=xT[kt * 128:(kt + 1) * 128, :],
                            )
                            for mt in range(8):
                                nc.tensor.matmul(
                                    pss[mt],
                                    w_t[:, mt * 128:(mt + 1) * 128],
                                    xqT_sb[:, kt, :],
                                    start=(kt == 0),
                                    stop=(kt == NKD - 1),
                                )
                        nc.sync.dma_start(
                            out=bqkp_sb,
                            in_=bqkp.rearrange("(t p) o -> p t o", p=128),
                        )
                        nc.sync.dma_start(
                            out=bvb, in_=_bcast_row(bv_row, 80 * H)
                        )
                        for mt in range(8):
                            nc.vector.tensor_scalar_add(
                                out=qT_sb[:, mt, :], in0=pss[mt],
                                scalar1=bqkp_sb[:, mt, :],
                            )

                        for g in range(2):
                            pss = [
                                [
                                    psA.tile(
                                        [128, LH], F32, tag="ps",
                                        name=f"psk{i}_{nq}",
                                    )
                                    for nq in range(2)
                                ]
                                for i in range(4)
                            ]
                            for kt in range(NKD):
                                w_t = wA.tile([128, LH], BF16, tag="wk")
                                nc.sync.dma_start(
                                    out=w_t,
                                    in_=wqkT[
                                        kt * 128:(kt + 1) * 128,
                                        MAXD + g * 512:MAXD + (g + 1) * 512,
                                    ],
                                )
                                for mi in range(4):
                                    for nq in range(2):
                                        nc.tensor.matmul(
                                            pss[mi][nq],
                                            w_t[:, mi * 128:(mi + 1) * 128],
                                            xT_sb[:, kt, nq * 512:(nq + 1) * 512],
                                            start=(kt == 0),
                                            stop=(kt == NKD - 1),
                                        )
                            for mi in range(4):
                                mt = g * 4 + mi
                                for nq in range(2):
                                    nc.vector.tensor_scalar_add(
                                        out=kT_sb[:, mt, nq * 512:(nq + 1) * 512],
                                        in0=pss[mi][nq],
                                        scalar1=bqkp_sb[:, 8 + mt, :],
                                    )

                        for c in range(4):
                            pss = [
                                psA.tile([128, 320], F32, tag="ps", name=f"psv{i}")
                                for i in range(8)
                            ]
                            for kt in range(NKD):
                                w_t = wA.tile([128, 320], BF16, tag="wv")
                                nc.sync.dma_start(
                                    out=w_t,
                                    in_=wvT[
                                        kt * 128:(kt + 1) * 128,
                                        c * 320:(c + 1) * 320,
                                    ],
                                )
                                for mtk in range(8):
                                    nc.tensor.matmul(
                                        pss[mtk],
                                        xT_sb[:, kt, mtk * 128:(mtk + 1) * 128],
                                        w_t,
                                        start=(kt == 0),
                                        stop=(kt == NKD - 1),
                                    )
                            for mtk in range(8):
                                nc.vector.tensor_add(
                                    out=vaug_sb[:, mtk, c * 320:(c + 1) * 320],
                                    in0=pss[mtk],
                                    in1=bvb[:, c * 320:(c + 1) * 320],
                                )

                # ======== SEQ B: attention ========
                with tc.tile_pool(name="seqot", bufs=1) as seqot:
                    OT_sb = seqot.tile([128, NKL, LH], BF16)
                    with (
                        tc.tile_pool(name="relbp", bufs=4) as relbp,
                        tc.tile_pool(name="sexp", bufs=8) as sexp,
                        tc.tile_pool(name="otn", bufs=4) as otn,
                        tc.tile_pool(name="psS", bufs=3, space="PSUM") as psS,
                        tc.tile_pool(name="psO", bufs=2, space="PSUM") as psO,
                    ):
                        for hpair in range(H // 2):
                            h0, h1 = 2 * hpair, 2 * hpair + 1
                            ht = hpair
                            strips = []
                            for h in (h0, h1):
                                strip = relbp.tile(
                                    [128, RELB_W], BF16, tag="strip",
                                    name=f"strip{h % 2}",
                                )
                                nc.sync.dma_start(out=strip, in_=relb[h])
                                strips.append(strip)
                            o_pss = [
                                psO.tile([65, LH], F32, tag="o", name=f"ops{i}")
                                for i in range(2)
                            ]
                            pend = None
                            for kd in range(4):
                                s_pss = [
                                    psS.tile(
                                        [128, 2 * LH], F32, tag="s",
                                        name=f"sps{i}",
                                    )
                                    for i in range(2)
                                ]
                                for i, hp in ((0, 0), (1, 64)):
                                    for j in range(2):
                                        k0 = 2 * kd + j
                                        nc.tensor.matmul(
                                            s_pss[i][:, j * LH:(j + 1) * LH],
                                            kT_sb[
                                                hp:hp + HD, ht,
                                                k0 * 128:(k0 + 1) * 128,
                                            ],
                                            qT_sb[hp:hp + HD, ht, :],
                                            start=True,
                                            stop=True,
                                        )
                                attns = []
                                for i in range(2):
                                    attn = sexp.tile(
                                        [128, 2 * LH], BF16, tag="attn",
                                        name=f"attn{i}",
                                    )
                                    nc.scalar.activation(
                                        out=attn, in_=s_pss[i], func=AF.Exp
                                    )
                                    for j in range(2):
                                        k0 = 2 * kd + j
                                        c0 = 896 - k0 * 128
                                        nc.vector.tensor_mul(
                                            out=attn[:, j * LH:(j + 1) * LH],
                                            in0=attn[:, j * LH:(j + 1) * LH],
                                            in1=strips[i][:, c0:c0 + LH],
                                        )
                                    attns.append(attn)
                                if pend is not None:
                                    pkd, pattns = pend
                                    for i, h in ((0, h0), (1, h1)):
                                        for j in range(2):
                                            pk0 = 2 * pkd + j
                                            nc.tensor.matmul(
                                                o_pss[i],
                                                vaug_sb[:, pk0, 80 * h:80 * h + 65],
                                                pattns[i][:, j * LH:(j + 1) * LH],
                                                start=(pk0 == 0),
                                                stop=False,
                                            )
                                pend = (kd, attns)
                            pkd, pattns = pend
                            for i, h in ((0, h0), (1, h1)):
                                for j in range(2):
                                    pk0 = 2 * pkd + j
                                    nc.tensor.matmul(
                                        o_pss[i],
                                        vaug_sb[:, pk0, 80 * h:80 * h + 65],
                                        pattns[i][:, j * LH:(j + 1) * LH],
                                        start=(pk0 == 0),
                                        stop=(pk0 == NKL - 1),
                                    )
                            for i, h in ((0, h0), (1, h1)):
                                hp = 64 * i
                                rs = otn.tile([1, LH], F32, tag="rs")
                                if i == 0:
                                    nc.scalar.activation(
                                        out=rs, in_=o_pss[i][64:65, :],
                                        func=AF.Ln,
                                    )
                                    nc.scalar.activation(
                                        out=rs, in_=rs, func=AF.Exp, scale=-1.0
                                    )
                                else:
                                    nc.vector.reciprocal(
                                        out=rs, in_=o_pss[i][64:65, :]
                                    )
                                bc_sb = otn.tile([HC, LH], F32, tag="bc_sb")
                                nc.gpsimd.partition_broadcast(bc_sb, rs)
                                nc.vector.tensor_mul(
                                    out=OT_sb[hp:hp + HDP, ht, :],
                                    in0=o_pss[i][0:HDP, :], in1=bc_sb,
                                )

                    # ======== SEQ C: out-proj + LN ========
                    with (
                        tc.tile_pool(name="xqs", bufs=1) as xqs,
                        tc.tile_pool(name="wC", bufs=6) as wC,
                        tc.tile_pool(name="psC", bufs=4, space="PSUM") as psC,
                    ):
                        xqseq_sb = xqs.tile([128, NLT, D], F32)
                        pss = [
                            psC.tile([128, D], F32, tag="op", name=f"psop{i}")
                            for i in range(NLT)
                        ]
                        for kt in range(NKL):
                            w_t = wC.tile([128, D], BF16, tag="wo")
                            nc.sync.dma_start(
                                out=w_t, in_=woT[kt * 128:(kt + 1) * 128, :]
                            )
                            for lt in range(NLT):
                                for n0, n1 in ((0, 512), (512, D)):
                                    nc.tensor.matmul(
                                        pss[lt][:, n0:n1],
                                        OT_sb[:, kt, lt * 128:(lt + 1) * 128],
                                        w_t[:, n0:n1],
                                        start=(kt == 0),
                                        stop=(kt == NKL - 1),
                                    )
                        nc.sync.dma_start(
                            out=xqseq_sb,
                            in_=xq_seq.rearrange("(t p) n -> p t n", p=128),
                        )
                        for lt in range(NLT):
                            t_sb = lntmp.tile([128, D], F32, tag="ln_t")
                            nc.vector.tensor_add(
                                out=t_sb, in0=pss[lt], in1=xqseq_sb[:, lt, :]
                            )
                            layernorm(
                                t_sb, g_seq_row, b_seq_row, xseq_sb[:, lt, :]
                            )

            # ======== CHAN consume: reduce-scattered half + LN ========
            with tc.tile_pool(name="cocons", bufs=1) as cocons:
                stage2 = cocons.tile([128, NLT, D], F32)
                nc.sync.dma_start(
                    out=stage2,
                    in_=copartRS.ap().rearrange("(t p) n -> p t n", p=128),
                )
                xqchan_sb = cocons.tile([128, NLT, D], F32)
                nc.sync.dma_start(
                    out=xqchan_sb,
                    in_=xq_chan.rearrange("(t p) n -> p t n", p=128),
                )
                for lt in range(NLT):
                    t_sb = lntmp.tile([128, D], F32, tag="ln_t")
                    nc.vector.tensor_add(
                        out=t_sb, in0=stage2[:, lt, :], in1=xqchan_sb[:, lt, :]
                    )
                    layernorm(t_sb, g_chan_row, b_chan_row, xchan_sb[:, lt, :])

            # ======== FFN + final LN ========
            with (
                tc.tile_pool(name="ffn", bufs=1) as ffn,
                tc.tile_pool(name="wE", bufs=6) as wE,
            ):
                fT_sb = ffn.tile([128, NFT, LH], BF16)
                b1_sb = smalls.tile([128, NFT, 1], F32)
                nc.sync.dma_start(
                    out=b1_sb, in_=b1col.rearrange("(t p) o -> p t o", p=128)
                )
                hT_sb = ffn.tile([128, NFT, LH], BF16)
                pre_res = ffn.tile([128, NLT, D], F32)
                if not skip_affine:
                    b2b = smalls.tile([128, D], F32)
                    nc.sync.dma_start(out=b2b, in_=_bcast_row(b2_row, D))
                with tc.tile_pool(name="psE", bufs=8, space="PSUM") as psE:
                    for ct in range(NFT):
                        for lt in range(NLT):
                            src = (
                                xseq_sb[:, lt, ct * 128:(ct + 1) * 128]
                                if ct < 6
                                else xchan_sb[:, lt, (ct - 6) * 128:(ct - 5) * 128]
                            )
                            tp = psE.tile([128, LH], F32, tag="ps", name="tp")
                            nc.tensor.matmul(
                                tp[:, 0:128], src, ident16, start=True, stop=True
                            )
                            nc.vector.tensor_copy(
                                out=fT_sb[:, ct, lt * 128:(lt + 1) * 128],
                                in_=tp[:, 0:128],
                            )

                    for g in range(2):
                        pss = [
                            psE.tile([128, LH], F32, tag="ps", name=f"psh{i}")
                            for i in range(6)
                        ]
                        for kt in range(NFT):
                            w_t = wE.tile([128, D], BF16, tag="w1")
                            nc.sync.dma_start(
                                out=w_t,
                                in_=w1T[
                                    kt * 128:(kt + 1) * 128, g * D:(g + 1) * D
                                ],
                            )
                            for mi in range(6):
                                nc.tensor.matmul(
                                    pss[mi],
                                    w_t[:, mi * 128:(mi + 1) * 128],
                                    fT_sb[:, kt, :],
                                    start=(kt == 0),
                                    stop=(kt == NFT - 1),
                                )
                        for mi in range(6):
                            mt = g * 6 + mi
                            nc.scalar.activation(
                                out=hT_sb[:, mt, :], in_=pss[mi], func=AF.Relu,
                                bias=b1_sb[:, mt, :],
                            )

                    for lt in range(NLT):
                        nc.vector.tensor_add(
                            out=pre_res[:, lt, :],
                            in0=xseq_sb[:, lt, :],
                            in1=xchan_sb[:, lt, :],
                        )
                        if not skip_affine:
                            nc.vector.tensor_add(
                                out=pre_res[:, lt, :], in0=pre_res[:, lt, :],
                                in1=b2b,
                            )

                    pss = [
                        [
                            psE.tile(
                                [128, LH], F32, tag="ps", name=f"psfo{i}_{nq}"
                            )
                            for nq in range(2)
                        ]
                        for i in range(NLT)
                    ]
                    for kt in range(NFT):
                        w_t = wE.tile([128, D], BF16, tag="w2")
                        nc.sync.dma_start(
                            out=w_t, in_=w2T[kt * 128:(kt + 1) * 128, :]
                        )
                        for lt in range(NLT):
                            for nq, (n0, n1) in enumerate(((0, 512), (512, D))):
                                nc.tensor.matmul(
                                    pss[lt][nq][:, 0:n1 - n0],
                                    hT_sb[:, kt, lt * 128:(lt + 1) * 128],
                                    w_t[:, n0:n1],
                                    start=(kt == 0),
                                    stop=(kt == NFT - 1),
                                )
                    for lt in range(NLT):
                        t_sb = lntmp.tile([128, D], F32, tag="ln_t")
                        for nq, (n0, n1) in enumerate(((0, 512), (512, D))):
                            nc.vector.tensor_add(
                                out=t_sb[:, n0:n1],
                                in0=pss[lt][nq][:, 0:n1 - n0],
                                in1=pre_res[:, lt, n0:n1],
                            )
                        o_sb = lntmp.tile([128, D], F32, tag="ln_o")
                        layernorm(t_sb, g_ffn_row, b_ffn_row, o_sb)
                        nc.sync.dma_start(
                            out=out_d[lt * 128:(lt + 1) * 128, :], in_=o_sb
                        )

    nc.compile()
    return nc


def _prep_inputs(inputs):
    bf16 = ml_dtypes.bfloat16
    x = np.asarray(inputs["x"], dtype=np.float32)
    wq = np.asarray(inputs["wq"], dtype=np.float32)
    bq = np.asarray(inputs["bq"], dtype=np.float32)
    wk = np.asarray(inputs["wk"], dtype=np.float32)
    bk = np.asarray(inputs["bk"], dtype=np.float32)
    wv = np.asarray(inputs["wv"], dtype=np.float32)
    bv = np.asarray(inputs["bv"], dtype=np.float32)
    wo = np.asarray(inputs["wo"], dtype=np.float32)
    bo = np.asarray(inputs["bo"], dtype=np.float32)
    rel_bias = np.asarray(inputs["rel_bias"], dtype=np.float32)
    ciw = np.asarray(inputs["chan_in_w"], dtype=np.float32)
    cib = np.asarray(inputs["chan_in_b"], dtype=np.float32)
    cow = np.asarray(inputs["chan_out_w"], dtype=np.float32)
    cob = np.asarray(inputs["chan_out_b"], dtype=np.float32)
    w1 = np.asarray(inputs["ffn_w1"], dtype=np.float32)
    b1 = np.asarray(inputs["ffn_b1"], dtype=np.float32)
    w2 = np.asarray(inputs["ffn_w2"], dtype=np.float32)
    b2 = np.asarray(inputs["ffn_b2"], dtype=np.float32)

    sc_s = 1.0 / np.sqrt(np.float32(HD))
    sc_c = 1.0 / np.sqrt(np.float32(HC))

    wqT_pad = np.zeros((D, MAXD), np.float32)
    wkT_pad = np.zeros((D, MAXD), np.float32)
    bq_pad = np.zeros((MAXD,), np.float32)
    bk_pad = np.zeros((MAXD,), np.float32)
    for h in range(H):
        wqT_pad[:, HDP * h:HDP * h + HD] = (wq[HD * h:HD * h + HD, :] * sc_s).T
        wkT_pad[:, HDP * h:HDP * h + HD] = wk[HD * h:HD * h + HD, :].T
        bq_pad[HDP * h:HDP * h + HD] = bq[HD * h:HD * h + HD] * sc_s
        bk_pad[HDP * h:HDP * h + HD] = bk[HD * h:HD * h + HD]
    wqkT = np.ascontiguousarray(
        np.concatenate([wqT_pad, wkT_pad], axis=1).astype(bf16)
    )
    bqkp = np.ascontiguousarray(np.concatenate([bq_pad, bk_pad])[:, None])

    wvT_aug = np.zeros((D, 80 * H), np.float32)
    bv_row = np.zeros((1, 80 * H), np.float32)
    for h in range(H):
        wvT_aug[:, 80 * h:80 * h + HD] = wv[HD * h:HD * h + HD, :].T
        bv_row[0, 80 * h:80 * h + HD] = bv[HD * h:HD * h + HD]
        bv_row[0, 80 * h + HC] = 1.0
    wvT_aug = wvT_aug.astype(bf16)

    woT_pad = np.zeros((MAXD, D), np.float32)
    for h in range(H):
        woT_pad[HDP * h:HDP * h + HD, :] = wo[:, HD * h:HD * h + HD].T
    woT_pad = woT_pad.astype(bf16)

    q_w = ciw[0:L] * sc_c
    k_w = ciw[L:2 * L]
    v_w = ciw[2 * L:3 * L]
    cib_q = cib[0:L] * sc_c
    cib_k = cib[L:2 * L]

    w1T = np.ascontiguousarray(w1.T.astype(bf16))
    w2T = np.ascontiguousarray(w2.T.astype(bf16))
    owT = np.ascontiguousarray(cow.T)

    g1 = np.ascontiguousarray(np.asarray(inputs["g_seq"], np.float32)[None, :])
    b1r = np.ascontiguousarray(np.asarray(inputs["b_seq"], np.float32)[None, :])
    g2 = np.ascontiguousarray(np.asarray(inputs["g_chan"], np.float32)[None, :])
    b2r = np.ascontiguousarray(np.asarray(inputs["b_chan"], np.float32)[None, :])
    g3 = np.ascontiguousarray(np.asarray(inputs["g_ffn"], np.float32)[None, :])
    b3r = np.ascontiguousarray(np.asarray(inputs["b_ffn"], np.float32)[None, :])

    # exp(bias) strips: exp(s + b) = exp(s) * exp(b) applied on gpsimd
    exp_rel = np.exp(rel_bias)
    relb_p = []
    ii = np.arange(128)[:, None]
    ff = np.arange(RELB_W)[None, :]
    for p in range(2):
        idx = ii - ff + (1919 - 512 * p)
        np.clip(idx, 0, 2 * MAXD - 2, out=idx)
        relb_p.append(np.ascontiguousarray(
            exp_rel[idx, :].transpose(2, 0, 1).astype(bf16)
        ))

    # per-pair-half chan head-group tensors
    wiT_p, cbqk_p, wvcT_p, cvb_p, woutT_p = [], [], [], [], []
    for p in range(2):
        hsl = slice(512 * p, 512 * p + 512)
        wiT_p.append(np.ascontiguousarray(
            np.concatenate([q_w.T[:, hsl], k_w.T[:, hsl]], axis=1).astype(bf16)
        ))
        cbqk_p.append(np.ascontiguousarray(
            np.concatenate([cib_q[hsl], cib_k[hsl]])[:, None]
        ))
        wvc = np.zeros((L, 520), np.float32)
        cvb = np.zeros((1, 520), np.float32)
        for hh in range(8):
            h = 8 * p + hh
            wvc[:, 65 * hh:65 * hh + HC] = v_w[HC * h:HC * h + HC, :].T
            cvb[0, 65 * hh:65 * hh + HC] = cib[2 * L + HC * h:2 * L + HC * h + HC]
            cvb[0, 65 * hh + HC] = 1.0
        wvcT_p.append(wvc.astype(bf16))
        cvb_p.append(cvb)
        woutT_p.append(np.ascontiguousarray(owT[hsl, :].astype(bf16)))

    in_maps = []
    for core in range(8):
        b, p = core // 2, core % 2
        sl = slice(512 * p, 512 * p + 512)
        xb = x[b]
        m = {
            "x": np.ascontiguousarray(xb.astype(bf16)),
            "xT": np.ascontiguousarray(xb.T.astype(bf16)),
            "xqT": np.ascontiguousarray(xb[sl].T.astype(bf16)),
            "xq_seq": np.ascontiguousarray(xb[sl] + bo[None, :]),
            "xq_chan": np.ascontiguousarray(xb[sl] + cob[sl][:, None]),
            "wqkT": wqkT,
            "bqkp": bqkp,
            "wvT": wvT_aug,
            "bv_row": bv_row,
            "woT": woT_pad,
            "relb": relb_p[p],
            "wiT": wiT_p[p],
            "cbqk": cbqk_p[p],
            "wvcT": wvcT_p[p],
            "cvb_row": cvb_p[p],
            "woutT": woutT_p[p],
            "w1T": w1T,
            "b1col": np.ascontiguousarray(b1[:, None]),
            "w2T": w2T,
            "b2_row": np.ascontiguousarray(b2[None, :]),
            "g_seq_row": g1, "b_seq_row": b1r,
            "g_chan_row": g2, "b_chan_row": b2r,
            "g_ffn_row": g3, "b_ffn_row": b3r,
            "ident16_in": np.eye(128, dtype=bf16),
        }
        in_maps.append(m)
    return in_maps


def kernel(**inputs) -> np.ndarray:
    in_maps = _prep_inputs(inputs)
    skip = all(
        np.all(np.asarray(inputs[g]) == 1.0) for g in ("g_seq", "g_chan", "g_ffn")
    ) and all(
        np.all(np.asarray(inputs[b]) == 0.0)
        for b in ("b_seq", "b_chan", "b_ffn", "ffn_b2")
    )
    key = ("nc", skip)
    if key not in _CACHE:
        _CACHE[key] = build(skip_affine=skip)
    res = run_bass_kernel_spmd(_CACHE[key], in_maps, core_ids=list(range(8)))
    out = np.empty((4, L, D), np.float32)
    for core in range(8):
        b, p = core // 2, core % 2
        out[b, 512 * p:512 * p + 512, :] = res.results[core]["out"]
    return out
